# revision 1
# baseline (speedup 1.0000x reference)
"""CSGNN Trainium kernel: host preprocessing + Bass/Tile kernel builder.

Data-parallel over graphs: nodes partitioned at graph boundaries across 8
cores, edges live on their dst node's core grouped by 128-node dst blocks.
Per layer: t = h @ W computed per-core, AllGathered, src rows fetched with
dma_gather (int16 indices, table split at 32768 rows), messages m = t[src]*e
aggregated with one-hot matmuls into PSUM.
"""
import numpy as np

import concourse.bacc as bacc
import concourse.bass as bass
import concourse.tile as tile
import concourse.mybir as mybir
from concourse import library_config

F32 = mybir.dt.float32
NC = 8
H = 128
P = 128
TSPLIT = 32768
MAX_GATHER_IDX = 1024  # per dma_gather call (HW hangs at 2048)


def _ceil(a, b):
    return -(-a // b)


def preprocess(x, edge_attr, edge_index, batch, n_graphs):
    """Compute the sharding plan + per-core host arrays."""
    x = np.asarray(x, np.float32)
    edge_attr = np.asarray(edge_attr, np.float32)
    ei = np.asarray(edge_index, np.int64)
    batch = np.asarray(batch, np.int64)
    N, F = x.shape
    E, Fe = edge_attr.shape
    G = n_graphs
    assert G % NC == 0
    gpc = G // NC  # graphs per core

    src, dst = ei[0], ei[1]

    # node partition at graph boundaries
    node_start = np.searchsorted(batch, np.arange(0, G + 1, gpc), side="left")
    n_local = np.diff(node_start)
    n_pad = max(_ceil(int(n_local.max()), P) * P, P)
    NB = n_pad // P

    # degrees / normalization (index-derived scalars)
    deg = 1.0 + np.bincount(dst, minlength=N).astype(np.float32)
    dis = 1.0 / np.sqrt(deg)
    norm = (dis[src] * dis[dst]).astype(np.float32)
    dis2 = (dis * dis).astype(np.float32)

    core_of = np.searchsorted(node_start[1:], np.arange(N), side="right")
    gidx = (core_of * n_pad + (np.arange(N) - node_start[core_of])).astype(np.int64)

    NR = NC * n_pad  # rows of the gathered table
    split = NR > TSPLIT
    gsrc = gidx[src]
    dst_core = core_of[dst]
    dst_local = dst - node_start[dst_core]
    blk = dst_local // P
    dst_in_blk = dst_local % P
    half = (gsrc >= TSPLIT).astype(np.int64) if split else np.zeros(E, np.int64)

    # bucket (core, block, half)
    key = (dst_core * NB + blk) * 2 + half
    order = np.argsort(key, kind="stable")
    cnt = np.bincount(key, minlength=NC * NB * 2).reshape(NC, NB, 2)

    T = _ceil(cnt.max(axis=0), P)  # [NB, 2] tiles per (block, half), shared by cores
    Tb = T.sum(axis=1)             # tiles per block
    n_tiles = int(Tb.sum())
    E_pad = n_tiles * P

    # slot offsets per (block, half) in the padded layout
    tile_off = np.zeros((NB, 2), np.int64)  # tile index where (b,h) starts
    run = 0
    for b in range(NB):
        for h in range(2):
            tile_off[b, h] = run
            run += T[b, h]
    assert run == n_tiles

    # gather call plan: per block, per half: calls of <= MAX_GATHER_IDX indices
    # entries: (block, half, tile_start_in_block, ntiles, idxcol_off)
    calls = []
    idxcols = 0
    for b in range(NB):
        for h in range(2):
            t0 = 0
            while t0 < T[b, h]:
                nt = min(MAX_GATHER_IDX // P, T[b, h] - t0)
                calls.append((b, h, int(tile_off[b, h] - tile_off[b, 0] + t0),
                              int(nt), idxcols))
                idxcols += nt * P // 16
                t0 += nt

    # per-core arrays
    cores = []
    counts_nodes = np.bincount(batch, minlength=G).astype(np.float32)
    for c in range(NC):
        sel = order[(dst_core[order] == c)]
        # slot assignment: within (block, half) real edges first
        kb = blk[sel]
        kh = half[sel]
        # position within bucket
        bucket_id = kb * 2 + kh
        # stable order -> cumulative position per bucket
        pos = np.zeros(len(sel), np.int64)
        bc = np.zeros(NB * 2, np.int64)
        # vectorized within-bucket position
        o2 = np.argsort(bucket_id, kind="stable")
        sb = bucket_id[o2]
        boundaries = np.searchsorted(sb, np.arange(NB * 2))
        poss = np.arange(len(sel)) - boundaries[sb]
        pos[o2] = poss
        slot = (tile_off[kb, kh] * P + pos).astype(np.int64)

        eaT = np.zeros((Fe, E_pad), np.float32)
        norm_f = np.zeros(E_pad, np.float32)
        dst_f = np.zeros(E_pad, np.float32)
        gi = np.zeros(E_pad, np.int64)
        eaT[:, slot] = edge_attr[sel].T
        norm_f[slot] = norm[sel]
        dst_f[slot] = dst_in_blk[sel].astype(np.float32)
        gi[slot] = gsrc[sel] - np.where(kh == 1, TSPLIT, 0)

        norm_col = np.ascontiguousarray(norm_f.reshape(n_tiles, P).T)
        dst_col = np.ascontiguousarray(dst_f.reshape(n_tiles, P).T)

        # wrapped int16 gather indices per call
        idx16 = np.zeros((P, idxcols), np.int16)
        for (b, h, ts, nt, co) in calls:
            t_global = tile_off[b, h] + (ts - (tile_off[b, h] - tile_off[b, 0]))
            vals = gi[t_global * P:(t_global + nt) * P].astype(np.int16)
            wrapped = vals.reshape(nt * P // 16, 16).T  # [16, ni/16]
            idx16[:, co:co + nt * P // 16] = np.tile(wrapped, (8, 1))

        # nodes
        ns, ne = node_start[c], node_start[c + 1]
        nl = ne - ns
        xT = np.zeros((F, n_pad), np.float32)
        xT[:, :nl] = x[ns:ne].T
        tmp = np.zeros(n_pad, np.float32)
        tmp[:nl] = dis2[ns:ne]
        dis2_col = np.ascontiguousarray(tmp.reshape(NB, P).T)
        tmp2 = -np.ones(n_pad, np.float32)
        tmp2[:nl] = (batch[ns:ne] - c * gpc).astype(np.float32)
        batch_col = np.ascontiguousarray(tmp2.reshape(NB, P).T)
        invc = np.zeros((P, 1), np.float32)
        invc[:gpc, 0] = 1.0 / np.maximum(counts_nodes[c * gpc:(c + 1) * gpc], 1.0)

        cores.append(dict(eaT=eaT, norm_col=norm_col, dst_col=dst_col,
                          idx16=idx16, xT=xT, dis2_col=dis2_col,
                          batch_col=batch_col, invc=invc))

    plan = dict(N=N, F=F, E=E, Fe=Fe, G=G, gpc=gpc, n_pad=n_pad, NB=NB,
                NR=NR, split=split, T=T, Tb=Tb, n_tiles=n_tiles,
                E_pad=E_pad, tile_off=tile_off, calls=calls, idxcols=idxcols,
                cores=cores)
    return plan


def build_kernel(plan, weights, n_layers, debug=False, sim1=False, bf16=False):
    """weights: dict of numpy arrays (full, unsharded)."""
    F, Fe, NB, n_pad = plan["F"], plan["Fe"], plan["NB"], plan["n_pad"]
    NR, n_tiles, E_pad = plan["NR"], plan["n_tiles"], plan["E_pad"]
    T, tile_off, calls, idxcols = plan["T"], plan["tile_off"], plan["calls"], plan["idxcols"]
    gpc = plan["gpc"]
    split = plan["split"]
    L = n_layers
    maxTb = int(plan["Tb"].max())
    DT = mybir.dt.bfloat16 if bf16 else F32

    nc = bacc.Bacc("TRN2", target_bir_lowering=False, debug=False,
                   num_devices=(1 if sim1 else NC), num_swdge_queues=4)

    def inp(name, shape):
        return nc.dram_tensor(name, list(shape), F32, kind="ExternalInput")

    d_eaT = inp("eaT", (Fe, E_pad))
    d_norm = inp("norm_col", (P, n_tiles))
    d_dst = inp("dst_col", (P, n_tiles))
    d_idx = nc.dram_tensor("idx16", [P, idxcols], mybir.dt.int16, kind="ExternalInput")
    d_xT = inp("xT", (F, n_pad))
    d_dis2 = inp("dis2_col", (P, NB))
    d_batch = inp("batch_col", (P, NB))
    d_invc = inp("invc", (P, 1))
    d_iota = inp("iota128", (P, P))
    d_iota32 = inp("iota32", (P, gpc))
    d_ident = inp("identity", (P, P))
    d_Wn = inp("W_node", (F, H))
    d_bn = inp("b_node", (1, H))
    d_We1 = inp("W_e1", (Fe, H))
    d_be1 = inp("b_e1", (H, 1))
    d_We2 = inp("W_e2", (H, H))
    d_be2 = inp("b_e2", (1, H))
    d_Wc = inp("W_convs", (L, H, H))
    d_bc = inp("b_convs", (L, H))
    d_Wl1 = inp("W_l1", (H, H))
    d_bl1 = inp("b_l1", (1, H))
    d_Wl2 = inp("W_l2", (H, 1))
    b_l2_val = float(np.asarray(weights["b_l2"]).reshape(-1)[0])
    d_out = nc.dram_tensor("out", [gpc, 1], F32, kind="ExternalOutput")
    if debug:
        d_dbg_h0 = nc.dram_tensor("dbg_h0", [P, NB * P], F32, kind="ExternalOutput")
        d_dbg_e = nc.dram_tensor("dbg_e", [E_pad, H], F32, kind="ExternalOutput")
        d_dbg_t0 = nc.dram_tensor("dbg_t0", [NR, H], F32, kind="ExternalOutput")
        d_dbg_h1 = nc.dram_tensor("dbg_h1", [P, NB * P], F32, kind="ExternalOutput")
        d_dbg_g = nc.dram_tensor("dbg_g", [gpc, H], F32, kind="ExternalOutput")
        d_dbg_gath = nc.dram_tensor("dbg_gath", [P, int(plan["Tb"].max()) * P], F32, kind="ExternalOutput")
        d_dbg_m = nc.dram_tensor("dbg_m", [P, int(plan["Tb"].max()) * P], F32, kind="ExternalOutput")

    CHUNK = 4  # edge-MLP tiles per chunk (512 edges)

    with tile.TileContext(nc) as tc:
        with tc.tile_pool(name="cst", bufs=1) as cst, \
             tc.tile_pool(name="big", bufs=1) as bigp, \
             tc.tile_pool(name="work", bufs=4) as work, \
             tc.tile_pool(name="ework", bufs=3) as ework, \
             tc.tile_pool(name="small", bufs=4) as small, \
             tc.tile_pool(name="psA", bufs=4, space="PSUM") as psA, \
             tc.tile_pool(name="psB", bufs=4, space="PSUM") as psB, \
             tc.tile_pool(name="dram", bufs=1, space="DRAM") as dram:

            nc.gpsimd.load_library(library_config.mlp)

            def load_const(dt_, shape, src_ap, dtype=F32):
                t = cst.tile(list(shape), dtype, tag=dt_)
                nc.sync.dma_start(out=t[:], in_=src_ap)
                return t

            iota_t = load_const("iota", (P, P), d_iota[:, :])
            iota32_t = load_const("iota32", (P, gpc), d_iota32[:, :])
            ident_t = load_const("ident", (P, P), d_ident[:, :])
            norm_t = load_const("norm", (P, n_tiles), d_norm[:, :])
            dst_t = load_const("dst", (P, n_tiles), d_dst[:, :])
            idx_t = load_const("idx", (P, idxcols), d_idx[:, :], mybir.dt.int16)
            xT_t = load_const("xT", (F, n_pad), d_xT[:, :])
            dis2_t = load_const("dis2", (P, NB), d_dis2[:, :])
            batch_t = load_const("batch", (P, NB), d_batch[:, :])
            invc_t = load_const("invc", (P, 1), d_invc[:, :])
            Wn_t = load_const("Wn", (F, H), d_Wn[:, :])
            bn_t = load_const("bn", (1, H), d_bn[:, :])
            We1_t = load_const("We1", (Fe, H), d_We1[:, :])
            be1_t = load_const("be1", (H, 1), d_be1[:, :])
            We2_t = load_const("We2", (H, H), d_We2[:, :])
            be2_t = load_const("be2", (1, H), d_be2[:, :])
            Wc_t = [load_const(f"Wc{l}", (H, H), d_Wc[l, :, :]) for l in range(L)]
            bc_t = [load_const(f"bc{l}", (1, H), d_bc[l:l + 1, :]) for l in range(L)]
            Wl1_t = load_const("Wl1", (H, H), d_Wl1[:, :])
            bl1_t = load_const("bl1", (1, H), d_bl1[:, :])
            Wl2_t = load_const("Wl2", (H, 1), d_Wl2[:, :])
            ones_t = cst.tile([1, 512], F32, tag="ones")
            nc.vector.memset(ones_t[:], 1.0)

            h_t = bigp.tile([P, NB * P], F32, tag="h")
            t_t = bigp.tile([P, NB * P], DT, tag="t")

            e_dram = dram.tile([E_pad, H], DT)
            t_loc = [dram.tile([n_pad, H], DT, name=f"t_loc{l}") for l in range(L)]
            t_full = [dram.tile([NR, H], DT,
                                addr_space=("Local" if sim1 else "Shared"),
                                name=f"t_full{l}")
                      for l in range(L)]

            AF = mybir.ActivationFunctionType

            # ---- edge MLP: e = relu(ea @ We1 + be1) @ We2 + be2 (eT orientation) ----
            def emit_edge_mlp():
              kk = 0
              while kk < n_tiles:
                  cw = min(CHUNK, n_tiles - kk)
                  w = cw * P
                  ea_t = ework.tile([Fe, CHUNK * P], F32, tag="ea")
                  nc.sync.dma_start(out=ea_t[:, :w], in_=d_eaT[:, kk * P:kk * P + w])
                  h1_ps = psB.tile([P, CHUNK * P], F32, tag="B")
                  nc.tensor.matmul(out=h1_ps[:, :w], lhsT=We1_t[:], rhs=ea_t[:, :w],
                                   start=True, stop=True)
                  h1_sb = ework.tile([P, CHUNK * P], F32, tag="h1")
                  nc.scalar.activation(out=h1_sb[:, :w], in_=h1_ps[:, :w],
                                       func=AF.Relu, bias=be1_t[:, 0:1])
                  eT_ps = psB.tile([P, CHUNK * P], F32, tag="B")
                  nc.tensor.matmul(out=eT_ps[:, :w], lhsT=We2_t[:], rhs=h1_sb[:, :w],
                                   start=True, stop=False)
                  nc.tensor.matmul(out=eT_ps[:, :w], lhsT=be2_t[:], rhs=ones_t[:, :w],
                                   start=False, stop=True)
                  eT_sb = ework.tile([P, CHUNK * P], F32, tag="eT")
                  nc.scalar.activation(out=eT_sb[:, :w], in_=eT_ps[:, :w], func=AF.Copy)
                  for t in range(cw):
                      e_ps = psA.tile([P, P], F32, tag="A")
                      nc.tensor.transpose(out=e_ps[:], in_=eT_sb[:, t * P:(t + 1) * P],
                                          identity=ident_t[:])
                      e_sb = small.tile([P, P], DT, tag="esb")
                      nc.scalar.activation(out=e_sb[:], in_=e_ps[:], func=AF.Copy)
                      nc.sync.dma_start(out=e_dram[(kk + t) * P:(kk + t + 1) * P, :],
                                        in_=e_sb[:])
                  kk += cw

            # ---- node embedding: h0 = x @ Wn + bn ----
            for b in range(NB):
                h0_ps = psA.tile([P, H], F32, tag="A")
                nc.tensor.matmul(out=h0_ps[:], lhsT=xT_t[:, b * P:(b + 1) * P],
                                 rhs=Wn_t[:], start=True, stop=False)
                nc.tensor.matmul(out=h0_ps[:], lhsT=ones_t[:, :P], rhs=bn_t[:],
                                 start=False, stop=True)
                nc.scalar.activation(out=h_t[:, b * P:(b + 1) * P], in_=h0_ps[:],
                                     func=AF.Copy)

            if debug:
                nc.sync.dma_start(out=d_dbg_h0[:, :], in_=h_t[:])
                nc.sync.dma_start(out=d_dbg_e[:, :], in_=e_dram[:, :])

            # ---- GCN layers ----
            for l in range(L):
                # phase A: t = h @ Wc[l]
                for b in range(NB):
                    bc0 = b * P
                    tr_ps = psB.tile([P, P], F32, tag="B")
                    nc.tensor.transpose(out=tr_ps[:], in_=h_t[:, bc0:bc0 + P],
                                        identity=ident_t[:])
                    hT_sb = small.tile([P, P], F32, tag="hT")
                    nc.vector.tensor_copy(hT_sb[:], tr_ps[:])
                    t_ps = psB.tile([P, P], F32, tag="B")
                    nc.tensor.matmul(out=t_ps[:], lhsT=hT_sb[:], rhs=Wc_t[l][:],
                                     start=True, stop=True)
                    nc.scalar.activation(out=t_t[:, bc0:bc0 + P], in_=t_ps[:],
                                         func=AF.Copy)
                    nc.sync.dma_start(out=t_loc[l][b * P:(b + 1) * P, :],
                                      in_=t_t[:, bc0:bc0 + P])
                # AllGather t
                if sim1:
                    nc.sync.dma_start(out=t_full[l][0:n_pad, :], in_=t_loc[l][:, :])
                    for _rr in range(1, NC):
                        nc.sync.dma_start(
                            out=t_full[l][_rr * n_pad:(_rr + 1) * n_pad, :],
                            in_=t_loc[l][:, :])
                else:
                    nc.gpsimd.collective_compute(
                        "AllGather", mybir.AluOpType.bypass,
                        replica_groups=[list(range(NC))],
                        ins=[t_loc[l][:]], outs=[t_full[l][:]])
                if l == 0:
                    emit_edge_mlp()
                if debug and l == 0:
                    nc.sync.dma_start(out=d_dbg_t0[:, :], in_=t_full[0][:, :])
                # phase B: aggregate per block
                qrot = 0
                for b in range(NB):
                    Tb = int(T[b, 0] + T[b, 1])
                    tile0 = int(tile_off[b, 0])
                    bc0 = b * P
                    g_t = work.tile([P, maxTb * P], DT, tag="g")
                    if Tb == 0:
                        agg_ps = psA.tile([P, H], F32, tag="A")
                        diag = small.tile([P, P], DT, tag="diag")
                        nc.vector.tensor_scalar(
                            out=diag[:], in0=ident_t[:],
                            scalar1=dis2_t[:, b:b + 1], scalar2=None,
                            op0=mybir.AluOpType.mult)
                        nc.tensor.matmul(out=agg_ps[:], lhsT=diag[:],
                                         rhs=t_t[:, bc0:bc0 + P],
                                         start=True, stop=False)
                        nc.tensor.matmul(out=agg_ps[:], lhsT=ones_t[:, :P],
                                         rhs=bc_t[l][:], start=False, stop=True)
                        nc.scalar.activation(out=h_t[:, bc0:bc0 + P], in_=agg_ps[:],
                                             func=AF.Relu)
                        continue
                    for (cb, ch, cts, cnt_, cco) in calls:
                        if cb != b:
                            continue
                        ni = cnt_ * P
                        src_ap = (t_full[l][0:TSPLIT, :] if (split and ch == 0)
                                  else (t_full[l][TSPLIT:NR, :] if split
                                        else t_full[l][0:NR, :]))
                        nc.gpsimd.dma_gather(
                            out_ap=g_t[:, cts * P:(cts + cnt_) * P]
                                .rearrange("p (j h) -> p j h", h=H),
                            in_ap=src_ap,
                            idxs_ap=idx_t[:, cco:cco + ni // 16],
                            num_idxs=ni, num_idxs_reg=ni, elem_size=H,
                            queue_num=qrot % 4)
                        qrot += 1
                    if debug and l == 0 and b == 0:
                        nc.sync.dma_start(out=d_dbg_gath[:, :], in_=g_t[:])
                    e_t = work.tile([P, maxTb * P], DT, tag="e")
                    nc.sync.dma_start(
                        out=e_t[:, :Tb * P].rearrange("p (t h) -> p t h", h=H),
                        in_=e_dram[tile0 * P:(tile0 + Tb) * P, :]
                            .rearrange("(t p) h -> p t h", p=P))
                    agg_ps = psA.tile([P, H], F32, tag="A")
                    for k in range(Tb):
                        oh = small.tile([P, P], DT, tag="oh", bufs=8)
                        nc.vector.tensor_scalar(
                            out=oh[:], in0=iota_t[:],
                            scalar1=dst_t[:, tile0 + k:tile0 + k + 1],
                            scalar2=norm_t[:, tile0 + k:tile0 + k + 1],
                            op0=mybir.AluOpType.is_equal,
                            op1=mybir.AluOpType.mult)
                        nc.vector.tensor_mul(
                            out=g_t[:, k * P:(k + 1) * P],
                            in0=g_t[:, k * P:(k + 1) * P],
                            in1=e_t[:, k * P:(k + 1) * P])
                        nc.tensor.matmul(out=agg_ps[:], lhsT=oh[:],
                                         rhs=g_t[:, k * P:(k + 1) * P],
                                         start=(k == 0), stop=False)
                    if debug and l == 0 and b == 0:
                        nc.sync.dma_start(out=d_dbg_m[:, :], in_=g_t[:])
                    diag = small.tile([P, P], DT, tag="diag")
                    nc.vector.tensor_scalar(
                        out=diag[:], in0=ident_t[:],
                        scalar1=dis2_t[:, b:b + 1], scalar2=None,
                        op0=mybir.AluOpType.mult)
                    nc.tensor.matmul(out=agg_ps[:], lhsT=diag[:],
                                     rhs=t_t[:, bc0:bc0 + P],
                                     start=False, stop=False)
                    nc.tensor.matmul(out=agg_ps[:], lhsT=ones_t[:, :P],
                                     rhs=bc_t[l][:], start=False, stop=True)
                    nc.scalar.activation(out=h_t[:, bc0:bc0 + P], in_=agg_ps[:],
                                         func=AF.Relu)

            if debug:
                nc.sync.dma_start(out=d_dbg_h1[:, :], in_=h_t[:])
            # ---- global mean pool ----
            g_ps = psA.tile([gpc, H], F32, tag="A")
            for b in range(NB):
                ohp = small.tile([P, gpc], F32, tag="ohp")
                nc.vector.tensor_scalar(
                    out=ohp[:], in0=iota32_t[:], scalar1=batch_t[:, b:b + 1],
                    scalar2=None, op0=mybir.AluOpType.is_equal)
                nc.tensor.matmul(out=g_ps[:], lhsT=ohp[:],
                                 rhs=h_t[:, b * P:(b + 1) * P],
                                 start=(b == 0), stop=(b == NB - 1))
            g_sb = small.tile([gpc, H], F32, tag="gsb")
            nc.vector.tensor_scalar(out=g_sb[:], in0=g_ps[:],
                                    scalar1=invc_t[:gpc, 0:1], scalar2=None,
                                    op0=mybir.AluOpType.mult)

            if debug:
                nc.sync.dma_start(out=d_dbg_g[:, :], in_=g_sb[:])
            # ---- head ----
            gT_ps = psB.tile([P, gpc], F32, tag="B")
            nc.tensor.transpose(out=gT_ps[:], in_=g_sb[:], identity=ident_t[:gpc, :gpc])
            gT_sb = small.tile([P, gpc], F32, tag="gT")
            nc.vector.tensor_copy(gT_sb[:], gT_ps[:])
            z1_ps = psB.tile([gpc, H], F32, tag="B")
            nc.tensor.matmul(out=z1_ps[:], lhsT=gT_sb[:], rhs=Wl1_t[:],
                             start=True, stop=False)
            nc.tensor.matmul(out=z1_ps[:], lhsT=ones_t[:, :gpc], rhs=bl1_t[:],
                             start=False, stop=True)
            z1_sb = small.tile([gpc, H], F32, tag="z1")
            nc.scalar.activation(out=z1_sb[:], in_=z1_ps[:], func=AF.Relu)
            z1T_ps = psB.tile([P, gpc], F32, tag="B")
            nc.tensor.transpose(out=z1T_ps[:], in_=z1_sb[:], identity=ident_t[:gpc, :gpc])
            z1T_sb = small.tile([P, gpc], F32, tag="z1T")
            nc.vector.tensor_copy(z1T_sb[:], z1T_ps[:])
            o2_ps = psA.tile([gpc, 1], F32, tag="A")
            nc.tensor.matmul(out=o2_ps[:], lhsT=z1T_sb[:], rhs=Wl2_t[:],
                             start=True, stop=True)
            out_sb = small.tile([gpc, 1], F32, tag="osb")
            nc.vector.tensor_scalar(out=out_sb[:], in0=o2_ps[:],
                                    scalar1=b_l2_val, scalar2=None,
                                    op0=mybir.AluOpType.add)
            nc.sync.dma_start(out=d_out[:, :], in_=out_sb[:])

    nc.compile()
    return nc


def make_in_maps(plan, weights, n_layers):
    L = n_layers
    iota128 = np.tile(np.arange(P, dtype=np.float32), (P, 1))
    iota32 = np.tile(np.arange(plan["gpc"], dtype=np.float32), (P, 1))
    ident = np.eye(P, dtype=np.float32)
    w = {k: np.asarray(v, np.float32) for k, v in weights.items()}
    shared = dict(
        iota128=iota128, iota32=np.ascontiguousarray(iota32), identity=ident,
        W_node=w["W_node"], b_node=w["b_node"].reshape(1, H),
        W_e1=w["W_e1"], b_e1=w["b_e1"].reshape(H, 1),
        W_e2=w["W_e2"], b_e2=w["b_e2"].reshape(1, H),
        W_convs=w["W_convs"].reshape(L, H, H),
        b_convs=w["b_convs"].reshape(L, H),
        W_l1=w["W_l1"], b_l1=w["b_l1"].reshape(1, H),
        W_l2=w["W_l2"].reshape(H, 1),
    )
    in_maps = []
    for c in range(NC):
        m = dict(shared)
        cc = plan["cores"][c]
        m.update(eaT=cc["eaT"], norm_col=cc["norm_col"], dst_col=cc["dst_col"],
                 idx16=cc["idx16"], xT=cc["xT"], dis2_col=cc["dis2_col"],
                 batch_col=cc["batch_col"], invc=cc["invc"])
        in_maps.append(m)
    return in_maps


# ----------------------------------------------------------------------------
# Public entry: kernel(**inputs) -> [256, 1] float32
# ----------------------------------------------------------------------------
N_GRAPHS = 256
N_LAYERS = 3


def _build_for_inputs(inputs):
    plan = preprocess(inputs["x"], inputs["edge_attr"], inputs["edge_index"],
                      inputs["batch"], N_GRAPHS)
    wkeys = ["W_node", "b_node", "W_e1", "b_e1", "W_e2", "b_e2", "W_convs",
             "b_convs", "W_l1", "b_l1", "W_l2", "b_l2"]
    w = {k: np.asarray(inputs[k], np.float32) for k in wkeys}
    nc_ = build_kernel(plan, w, N_LAYERS)
    in_maps = make_in_maps(plan, w, N_LAYERS)
    return nc_, in_maps, plan


def kernel(**inputs):
    from concourse.bass_utils import run_bass_kernel_spmd
    inputs = {k: np.asarray(v) for k, v in inputs.items()}
    nc_, in_maps, plan = _build_for_inputs(inputs)
    res = run_bass_kernel_spmd(nc_, in_maps, core_ids=list(range(NC)))
    out = np.concatenate([res.results[c]["out"] for c in range(NC)], axis=0)
    return out.astype(np.float32)



# revision 34
# speedup vs baseline: 1.6387x; 1.6387x over previous
"""CSGNN Trainium kernel: host preprocessing + Bass/Tile kernel builder.

Data-parallel over graphs: nodes partitioned at graph boundaries across 8
cores, edges live on their dst node's core grouped by 128-node dst blocks.
Per layer: t = h @ W computed per-core in bf16, AllGathered in 2 row-chunks
(each chunk table < 32768 rows so int16 gather indices address it directly,
and chunk-0 gathers overlap the chunk-1 collective), src rows fetched with
dma_gather, messages m = t[src]*(e*norm) aggregated with one-hot matmuls
into a per-superblock PSUM tile. e = MLP(edge_attr)*norm is stored once in
a partition-major tiled bf16 layout so per-layer reloads use large
contiguous descriptors.
"""
import numpy as np
import ml_dtypes

import concourse.bacc as bacc
import concourse.bass as bass
import concourse.tile as tile
import concourse.mybir as mybir
from concourse import library_config

F32 = mybir.dt.float32
BF16 = mybir.dt.bfloat16
NPBF16 = ml_dtypes.bfloat16
NC = 8
H = 128
P = 128
CH = 2            # AllGather chunks per layer
SB = 4            # dst blocks per superblock (gather/DVE batch unit)
LOOK = 3          # superblock gather lookahead across the chunk barrier
MAX_GATHER_IDX = 1024  # per dma_gather call (HW hangs at 2048)
PHASING = True    # manual tile-scheduler phase timestamps


def _ceil(a, b):
    return -(-a // b)


def preprocess(x, edge_attr, edge_index, batch, n_graphs):
    """Compute the sharding plan + per-core host arrays."""
    x = np.asarray(x, np.float32)
    edge_attr = np.asarray(edge_attr, np.float32)
    ei = np.asarray(edge_index, np.int64)
    batch = np.asarray(batch, np.int64)
    N, F = x.shape
    E, Fe = edge_attr.shape
    G = n_graphs
    assert G % NC == 0
    gpc = G // NC  # graphs per core

    src, dst = ei[0], ei[1]

    # node partition at graph boundaries
    node_start = np.searchsorted(batch, np.arange(0, G + 1, gpc), side="left")
    n_local = np.diff(node_start)
    n_pad = max(_ceil(int(n_local.max()), CH * P) * CH * P, CH * P)
    NB = n_pad // P
    ch_rows = n_pad // CH          # rows per AllGather chunk (per core)
    bpc = NB // CH                 # blocks per chunk
    assert NC * ch_rows < 32768    # int16 gather indices per chunk table

    # degrees / normalization (index-derived scalars)
    deg = 1.0 + np.bincount(dst, minlength=N).astype(np.float32)
    dis = 1.0 / np.sqrt(deg)

    core_of = np.searchsorted(node_start[1:], np.arange(N), side="right")
    loc = np.arange(N) - node_start[core_of]

    src_core = core_of[src]
    src_loc = loc[src]
    half = src_loc // ch_rows                       # AG chunk of the src row
    tbl_row = src_core * ch_rows + (src_loc - half * ch_rows)

    dst_core = core_of[dst]
    dst_local = dst - node_start[dst_core]
    blk = dst_local // P
    dst_in_blk = dst_local % P

    # bucket (core, block, half); tile counts shared across cores (SPMD)
    key = (dst_core * NB + blk) * CH + half
    cnt = np.bincount(key, minlength=NC * NB * CH).reshape(NC, NB, CH)
    T = _ceil(cnt.max(axis=0), P)   # [NB, CH] tiles per (block, half)

    NSB = _ceil(NB, SB)
    sb_blocks = [list(range(s * SB, min((s + 1) * SB, NB))) for s in range(NSB)]

    # global tile order: superblock-major, then half, then block
    tile_off = np.zeros((NB, CH), np.int64)
    sb_tile0 = np.zeros(NSB, np.int64)      # first tile of each superblock
    sb_ta = np.zeros(NSB, np.int64)         # chunk-0 tiles in superblock
    sb_tt = np.zeros(NSB, np.int64)         # total tiles in superblock
    run = 0
    for s in range(NSB):
        sb_tile0[s] = run
        for h in range(CH):
            for b in sb_blocks[s]:
                tile_off[b, h] = run
                run += T[b, h]
            if h == 0:
                sb_ta[s] = run - sb_tile0[s]
        sb_tt[s] = run - sb_tile0[s]
    n_tiles = int(run)
    E_pad = n_tiles * P

    # gather call plan: per (superblock, half): calls of <= MAX_GATHER_IDX
    # entries: (sb, half, tile_start_within_sb, ntiles, idxcol_off)
    calls = []
    idxcols = 0
    for s in range(NSB):
        for h in range(CH):
            nt_h = int(sum(T[b, h] for b in sb_blocks[s]))
            ts0 = 0 if h == 0 else int(sb_ta[s])
            t0 = 0
            while t0 < nt_h:
                nt = min(MAX_GATHER_IDX // P, nt_h - t0)
                calls.append((s, h, ts0 + t0, int(nt), idxcols))
                idxcols += nt * P // 16
                t0 += nt

    # per-core arrays
    order = np.argsort(key, kind="stable")
    cores = []
    counts_nodes = np.bincount(batch, minlength=G).astype(np.float32)
    for c in range(NC):
        sel = order[(dst_core[order] == c)]
        kb = blk[sel]
        kh = half[sel]
        bucket_id = kb * CH + kh
        # within-bucket position (stable order)
        pos = np.zeros(len(sel), np.int64)
        o2 = np.argsort(bucket_id, kind="stable")
        sb_sorted = bucket_id[o2]
        boundaries = np.searchsorted(sb_sorted, np.arange(NB * CH))
        pos[o2] = np.arange(len(sel)) - boundaries[sb_sorted]
        slot = (tile_off[kb, kh] * P + pos).astype(np.int64)

        eaT = np.zeros((Fe, E_pad), NPBF16)
        dst_f = np.zeros(E_pad, np.float32)
        gi = np.zeros(E_pad, np.int64)
        eaT[:, slot] = edge_attr[sel].T.astype(NPBF16)
        dst_f[slot] = dst_in_blk[sel].astype(np.float32)
        gi[slot] = tbl_row[sel]

        dst_col = np.ascontiguousarray(
            dst_f.reshape(n_tiles, P).T.astype(NPBF16))

        # wrapped int16 gather indices per call
        idx16 = np.zeros((P, idxcols), np.int16)
        for (s, h, ts, nt, co) in calls:
            t_global = int(sb_tile0[s]) + ts
            vals = gi[t_global * P:(t_global + nt) * P].astype(np.int16)
            wrapped = vals.reshape(nt * P // 16, 16).T  # [16, ni/16]
            idx16[:, co:co + nt * P // 16] = np.tile(wrapped, (8, 1))

        # nodes
        ns, ne = node_start[c], node_start[c + 1]
        nl = ne - ns
        xT = np.zeros((F, n_pad), NPBF16)
        xT[:, :nl] = x[ns:ne].T.astype(NPBF16)
        tmp = np.zeros(n_pad, np.float32)
        tmp[:nl] = dis[ns:ne]
        dis_col = np.ascontiguousarray(tmp.reshape(NB, P).T)
        tmp2 = -np.ones(n_pad, np.float32)
        tmp2[:nl] = (batch[ns:ne] - c * gpc).astype(np.float32)
        batch_col = np.ascontiguousarray(
            tmp2.reshape(NB, P).T.astype(NPBF16))
        invc = np.zeros((P, 1), np.float32)
        invc[:gpc, 0] = 1.0 / np.maximum(counts_nodes[c * gpc:(c + 1) * gpc], 1.0)

        cores.append(dict(eaT=eaT, dst_col=dst_col,
                          idx16=idx16, xT=xT, dis_col=dis_col,
                          batch_col=batch_col, invc=invc))

    plan = dict(N=N, F=F, E=E, Fe=Fe, G=G, gpc=gpc, n_pad=n_pad, NB=NB,
                ch_rows=ch_rows, bpc=bpc, NSB=NSB, sb_blocks=sb_blocks,
                T=T, tile_off=tile_off, sb_tile0=sb_tile0, sb_ta=sb_ta,
                sb_tt=sb_tt, n_tiles=n_tiles, E_pad=E_pad, calls=calls,
                idxcols=idxcols, cores=cores)
    return plan


def build_kernel(plan, weights, n_layers, debug=False):
    """weights: dict of numpy arrays (full, unsharded)."""
    F, Fe, NB, n_pad = plan["F"], plan["Fe"], plan["NB"], plan["n_pad"]
    n_tiles, E_pad = plan["n_tiles"], plan["E_pad"]
    T, tile_off, calls, idxcols = (plan["T"], plan["tile_off"], plan["calls"],
                                   plan["idxcols"])
    gpc, ch_rows, bpc = plan["gpc"], plan["ch_rows"], plan["bpc"]
    NSB, sb_blocks = plan["NSB"], plan["sb_blocks"]
    sb_tile0, sb_ta, sb_tt = plan["sb_tile0"], plan["sb_ta"], plan["sb_tt"]
    L = n_layers
    maxTa = int(max(sb_ta))
    maxTb = int(max(t - a for t, a in zip(sb_tt, sb_ta)))
    NRC = NC * ch_rows  # rows per chunk table

    nc = bacc.Bacc("TRN2", target_bir_lowering=False, debug=False,
                   num_devices=NC, num_swdge_queues=4)

    def inp(name, shape, dtype=F32):
        return nc.dram_tensor(name, list(shape), dtype, kind="ExternalInput")

    d_eaT = inp("eaT", (Fe, E_pad), BF16)
    d_dst = inp("dst_col", (P, n_tiles), BF16)
    d_idx = inp("idx16", (P, idxcols), mybir.dt.int16)
    d_xT = inp("xT", (F, n_pad), BF16)
    d_dis = inp("dis_col", (P, NB))
    d_batch = inp("batch_col", (P, NB), BF16)
    d_invc = inp("invc", (P, 1))
    d_iota = inp("iota128", (P, P), BF16)
    d_iota32 = inp("iota32", (P, gpc), BF16)
    d_ident = inp("identity", (P, P), BF16)
    d_identf = inp("identityf", (P, P))
    d_Wn = inp("W_node", (F, H), BF16)
    d_bn = inp("b_node", (1, H))
    d_We1 = inp("W_e1", (Fe, H), BF16)
    d_be1 = inp("b_e1", (H, 1))
    d_We2 = inp("W_e2", (H, H), BF16)
    d_be2 = inp("b_e2", (1, H))
    d_Wc = inp("W_convs", (L, H, H), BF16)
    d_bc = inp("b_convs", (L, H))
    d_bcb = inp("b_convs_bcast", (P, L * H))
    d_Wl1 = inp("W_l1", (H, H))
    d_bl1 = inp("b_l1", (1, H))
    d_Wl2 = inp("W_l2", (H, 1))
    b_l2_val = float(np.asarray(weights["b_l2"]).reshape(-1)[0])
    d_out = nc.dram_tensor("out", [gpc, 1], F32, kind="ExternalOutput")
    if debug:
        d_dbg_h0 = nc.dram_tensor("dbg_h0", [P, NB * P], BF16, kind="ExternalOutput")
        d_dbg_e = nc.dram_tensor("dbg_e", [P, n_tiles * H], BF16, kind="ExternalOutput")
        d_dbg_t0 = nc.dram_tensor("dbg_t0", [CH * NRC, H], BF16, kind="ExternalOutput")
        d_dbg_h1 = nc.dram_tensor("dbg_h1", [P, NB * P], BF16, kind="ExternalOutput")
        d_dbg_g0 = nc.dram_tensor("dbg_g0", [P, 4 * P], F32, kind="ExternalOutput")
        d_dbg_oh = nc.dram_tensor("dbg_oh", [P, 4 * P], F32, kind="ExternalOutput")
        d_dbg_part = nc.dram_tensor("dbg_part", [P, SB * H], BF16, kind="ExternalOutput")

    CHUNK = 4  # edge-MLP tiles per chunk (512 edges)

    with tile.TileContext(nc) as tc:
        with tc.tile_pool(name="cst", bufs=1) as cst, \
             tc.tile_pool(name="big", bufs=1) as bigp, \
             tc.tile_pool(name="ga", bufs=3) as gapool, \
             tc.tile_pool(name="gb", bufs=3) as gbpool, \
             tc.tile_pool(name="ea", bufs=3) as eapool, \
             tc.tile_pool(name="eb", bufs=3) as ebpool, \
             tc.tile_pool(name="oha", bufs=2) as ohapool, \
             tc.tile_pool(name="ohb", bufs=2) as ohbpool, \
             tc.tile_pool(name="part", bufs=NSB + 1) as partpool, \
             tc.tile_pool(name="ework", bufs=3) as ework, \
             tc.tile_pool(name="small", bufs=4) as small, \
             tc.tile_pool(name="psA", bufs=2, space="PSUM") as psA, \
             tc.tile_pool(name="psB", bufs=2, space="PSUM") as psB, \
             tc.tile_pool(name="psE", bufs=1, space="PSUM") as psE, \
             tc.tile_pool(name="psT", bufs=3, space="PSUM") as psT, \
             tc.tile_pool(name="dram", bufs=1, space="DRAM") as dram:

            nc.gpsimd.load_library(library_config.mlp)

            def load_const(dt_, shape, src_ap, dtype=F32):
                t = cst.tile(list(shape), dtype, tag=dt_)
                nc.sync.dma_start(out=t[:], in_=src_ap)
                return t

            iota_t = load_const("iota", (P, P), d_iota[:, :], BF16)
            iota32_t = load_const("iota32", (P, gpc), d_iota32[:, :], BF16)
            ident_t = load_const("ident", (P, P), d_ident[:, :], BF16)
            identf_t = load_const("identf", (P, P), d_identf[:, :])
            dst_t = load_const("dst", (P, n_tiles), d_dst[:, :], BF16)
            idx_t = load_const("idx", (P, idxcols), d_idx[:, :], mybir.dt.int16)
            dis_t = load_const("dis", (P, NB), d_dis[:, :])
            batch_t = load_const("batch", (P, NB), d_batch[:, :], BF16)
            invc_t = load_const("invc", (P, 1), d_invc[:, :])
            Wn_t = load_const("Wn", (F, H), d_Wn[:, :], BF16)
            bn_t = load_const("bn", (1, H), d_bn[:, :])
            We1_t = load_const("We1", (Fe, H), d_We1[:, :], BF16)
            be1_t = load_const("be1", (H, 1), d_be1[:, :])
            We2_t = load_const("We2", (H, H), d_We2[:, :], BF16)
            be2_t = load_const("be2", (1, H), d_be2[:, :])
            Wc_t = [load_const(f"Wc{l}", (H, H), d_Wc[l, :, :], BF16)
                    for l in range(L)]
            bcb_t = load_const("bcb", (P, L * H), d_bcb[:, :])
            bc_t = [load_const(f"bc{l}", (1, H), d_bc[l:l + 1, :])
                    for l in range(L)]
            Wl1_t = load_const("Wl1", (H, H), d_Wl1[:, :])
            bl1_t = load_const("bl1", (1, H), d_bl1[:, :])
            Wl2_t = load_const("Wl2", (H, 1), d_Wl2[:, :])
            ones_t = cst.tile([1, 512], F32, tag="ones")
            nc.vector.memset(ones_t[:], 1.0)
            zeros_t = cst.tile([1, P], F32, tag="zeros")
            nc.vector.memset(zeros_t[:], 0.0)

            h_t = bigp.tile([P, NB * P], BF16, tag="h")
            t_t = bigp.tile([P, NB * P], BF16, tag="t")

            # per-(superblock, half) e regions: loads wait only their own
            # region's stores (DRAM deps are tile-granular)
            e_reg = []
            for s in range(NSB):
                ta = int(sb_ta[s])
                tb = int(sb_tt[s]) - ta
                ra = dram.tile([P, max(ta, 1) * H], BF16, name=f"edrA{s}")
                rb = dram.tile([P, max(tb, 1) * H], BF16, name=f"edrB{s}")
                e_reg.append((ra, rb))
            t_locA = [dram.tile([ch_rows, H], BF16, name=f"t_locA{l}")
                      for l in range(L)]
            t_locB = [dram.tile([ch_rows, H], BF16, name=f"t_locB{l}")
                      for l in range(L)]
            t_full = [[dram.tile([NRC, H], BF16, addr_space="Shared",
                                 name=f"t_full{l}_{h}") for h in range(CH)]
                      for l in range(L)]

            AF = mybir.ActivationFunctionType

            # ---- edge MLP: e = relu(ea@We1+be1) @ We2 + be2  (no norm) ----
            mlp_ctr = [0]

            def emit_edge_mlp(e_reg):
                regions = []
                for s in range(NSB):
                    ta = int(sb_ta[s])
                    tb = int(sb_tt[s]) - ta
                    t0 = int(sb_tile0[s])
                    regions.append((t0, ta, e_reg[s][0]))
                    regions.append((t0 + ta, tb, e_reg[s][1]))
                DC = 12  # tiles per DMA chunk (compute stays CHUNK tiles)
                for (r0, rn, rtile) in regions:
                  dk = 0
                  while dk < rn:
                    dw = min(DC, rn - dk)
                    ea_t = ework.tile([Fe, DC * P], BF16, tag="ea", bufs=2)
                    nc.sync.dma_start(
                        out=ea_t[:, :dw * P],
                        in_=d_eaT[:, (r0 + dk) * P:(r0 + dk + dw) * P])
                    es = ework.tile([P, DC * H], BF16, tag="es", bufs=2)
                    kk = 0
                    while kk < dw:
                        cw = min(CHUNK, dw - kk)
                        w = cw * P
                        h1_ps = psB.tile([P, CHUNK * P], F32, tag="B")
                        nc.tensor.matmul(out=h1_ps[:, :w], lhsT=We1_t[:],
                                         rhs=ea_t[:, kk * P:kk * P + w],
                                         start=True, stop=True)
                        h1_sb = ework.tile([P, CHUNK * P], BF16, tag="h1",
                                           bufs=2)
                        use_act = (mlp_ctr[0] % 3 == 0)
                        mlp_ctr[0] += 1
                        if use_act:
                            nc.scalar.activation(
                                out=h1_sb[:, :w], in_=h1_ps[:, :w],
                                func=AF.Relu, bias=be1_t[:, 0:1])
                        else:
                            nc.vector.tensor_scalar(
                                out=h1_sb[:, :w], in0=h1_ps[:, :w],
                                scalar1=be1_t[:, 0:1], scalar2=0.0,
                                op0=mybir.AluOpType.add,
                                op1=mybir.AluOpType.max)
                        eT_ps = psB.tile([P, CHUNK * P], F32, tag="B")
                        nc.tensor.matmul(out=eT_ps[:, :w], lhsT=We2_t[:],
                                         rhs=h1_sb[:, :w],
                                         start=True, stop=False)
                        nc.tensor.matmul(out=eT_ps[:, :w], lhsT=be2_t[:],
                                         rhs=ones_t[:, :w],
                                         start=False, stop=True)
                        eT_sb = ework.tile([P, CHUNK * P], BF16, tag="eT",
                                           bufs=2)
                        if use_act:
                            nc.scalar.activation(out=eT_sb[:, :w],
                                                 in_=eT_ps[:, :w],
                                                 func=AF.Copy)
                        else:
                            nc.vector.tensor_copy(eT_sb[:, :w], eT_ps[:, :w])
                        e4_ps = psE.tile([P, CHUNK * P], BF16, tag="E")
                        for t in range(cw):
                            nc.tensor.transpose(
                                out=e4_ps[:, t * P:(t + 1) * P],
                                in_=eT_sb[:, t * P:(t + 1) * P],
                                identity=ident_t[:])
                        nc.scalar.activation(
                            out=es[:, kk * H:kk * H + cw * H],
                            in_=e4_ps[:, :cw * P], func=AF.Copy)
                        kk += cw
                    nc.scalar.dma_start(
                        out=rtile[:, dk * H:(dk + dw) * H],
                        in_=es[:, :dw * H])
                    dk += dw

            # ---- node embedding: h0 = x @ Wn + bn ----
            for b in range(NB):
                xb = small.tile([F, P], BF16, tag="xb")
                nc.sync.dma_start(out=xb[:], in_=d_xT[:, b * P:(b + 1) * P])
                h0_ps = psT.tile([P, H], F32, tag="T")
                nc.tensor.matmul(out=h0_ps[:], lhsT=xb[:],
                                 rhs=Wn_t[:], start=True, stop=False)
                nc.tensor.matmul(out=h0_ps[:], lhsT=ones_t[:, :P], rhs=bn_t[:],
                                 start=False, stop=True)
                nc.scalar.activation(out=h_t[:, b * P:(b + 1) * P],
                                     in_=h0_ps[:], func=AF.Copy)

            if debug:
                nc.sync.dma_start(out=d_dbg_h0[:, :], in_=h_t[:])

            # ---- GCN layers ----
            for l in range(L):
                # manual scheduler phasing: per-engine instruction order
                # follows these logical timestamps (ms units are logical)
                tc.tile_set_cur_wait(3 * l, enable=PHASING)
                # phase A: t' = (h @ Wc[l]) * dis ; AllGather per row-chunk
                for b in range(NB):
                    bc0 = b * P
                    tr_ps = psT.tile([P, P], BF16, tag="T")
                    nc.tensor.transpose(out=tr_ps[:], in_=h_t[:, bc0:bc0 + P],
                                        identity=ident_t[:])
                    hT_sb = small.tile([P, P], BF16, tag="hT")
                    nc.scalar.activation(out=hT_sb[:], in_=tr_ps[:],
                                         func=AF.Copy)
                    t_ps = psT.tile([P, P], F32, tag="T")
                    nc.tensor.matmul(out=t_ps[:], lhsT=hT_sb[:], rhs=Wc_t[l][:],
                                     start=True, stop=True)
                    nc.scalar.activation(out=t_t[:, bc0:bc0 + P], in_=t_ps[:],
                                         func=AF.Copy,
                                         scale=dis_t[:, b:b + 1])
                    tdst = t_locA[l] if b < bpc else t_locB[l]
                    boff = (b if b < bpc else b - bpc) * P
                    nc.scalar.dma_start(out=tdst[boff:boff + P, :],
                                        in_=t_t[:, bc0:bc0 + P])
                    if (b + 1) % bpc == 0:
                        h_ = (b + 1) // bpc - 1
                        nc.gpsimd.collective_compute(
                            "AllGather", mybir.AluOpType.bypass,
                            replica_groups=[list(range(NC))],
                            ins=[(t_locA[l] if h_ == 0 else t_locB[l])[:, :]],
                            outs=[t_full[l][h_][:]])

                if l == 0:
                    with tc.high_priority():
                        emit_edge_mlp(e_reg)

                # Pool gather streams: all chunk-0 calls, then all chunk-1
                tc.tile_set_cur_wait(3 * l + 1, enable=PHASING)
                ga_tiles, gb_tiles = {}, {}
                for s in range(NSB):
                    ga_tiles[s] = gapool.tile([P, maxTa * P], BF16,
                                              tag="ga", name=f"ga{l}_{s}")
                    qrot = 0
                    for (cs, chh, cts, cnt_, cco) in calls:
                        if cs != s or chh != 0:
                            continue
                        ni = cnt_ * P
                        nc.gpsimd.dma_gather(
                            out_ap=ga_tiles[s][:, cts * P:(cts + cnt_) * P]
                                .rearrange("p (j h) -> p j h", h=H),
                            in_ap=t_full[l][0][0:NRC, :],
                            idxs_ap=idx_t[:, cco:cco + ni // 16],
                            num_idxs=ni, num_idxs_reg=ni, elem_size=H,
                            queue_num=qrot % 4)
                        qrot += 1
                tc.tile_set_cur_wait(3 * l + 2, enable=PHASING)
                for s in range(NSB):
                    gb_tiles[s] = gbpool.tile([P, maxTb * P], BF16,
                                              tag="gb", name=f"gb{l}_{s}")
                    qrot = 2
                    for (cs, chh, cts, cnt_, cco) in calls:
                        if cs != s or chh != 1:
                            continue
                        ta = int(sb_ta[s])
                        ni = cnt_ * P
                        nc.gpsimd.dma_gather(
                            out_ap=gb_tiles[s][:, (cts - ta) * P:
                                               (cts - ta + cnt_) * P]
                                .rearrange("p (j h) -> p j h", h=H),
                            in_ap=t_full[l][1][0:NRC, :],
                            idxs_ap=idx_t[:, cco:cco + ni // 16],
                            num_idxs=ni, num_idxs_reg=ni, elem_size=H,
                            queue_num=qrot % 4)
                        qrot += 1

                if debug and l == 0:
                    for h_ in range(CH):
                        nc.sync.dma_start(
                            out=d_dbg_t0[h_ * NRC:(h_ + 1) * NRC, :],
                            in_=t_full[l][h_][:, :])

                # a-phase: messages from chunk-0 srcs -> partial sums
                tc.tile_set_cur_wait(3 * l + 1, enable=PHASING)
                partials = {}
                a_started = {}
                for s in range(NSB):
                    blocks = sb_blocks[s]
                    t0 = int(sb_tile0[s])
                    ta = int(sb_ta[s])
                    ga = ga_tiles[s]
                    agg_ps = psA.tile([P, SB * H], F32, tag="A")
                    started = [False] * len(blocks)
                    if ta > 0:
                        ea_e = eapool.tile([P, maxTa * H], BF16, tag="e")
                        nc.sync.dma_start(
                            out=ea_e[:, :ta * H],
                            in_=e_reg[s][0][:, :ta * H])
                        oh = ohapool.tile([P, maxTa * P], BF16, tag="oh")
                        nc.vector.tensor_tensor(
                            out=oh[:, :ta * P]
                                .rearrange("p (k c) -> p k c", c=P),
                            in0=iota_t[:].unsqueeze(1).broadcast_to([P, ta, P]),
                            in1=dst_t[:, t0:t0 + ta].unsqueeze(2)
                                .broadcast_to([P, ta, P]),
                            op=mybir.AluOpType.is_equal)
                        if debug and l == 0 and s == 0:
                            ohf = bigp.tile([P, 4 * P], F32, tag="ohf")
                            nc.vector.tensor_copy(ohf[:], oh[:, :4 * P])
                            nc.sync.dma_start(out=d_dbg_oh[:, :], in_=ohf[:])
                        nc.vector.tensor_mul(
                            out=ga[:, :ta * P], in0=ga[:, :ta * P],
                            in1=ea_e[:, :ta * H])
                        k = 0
                        for bi, b in enumerate(blocks):
                            nt_b = int(T[b, 0])
                            for j in range(nt_b):
                                nc.tensor.matmul(
                                    out=agg_ps[:, bi * H:(bi + 1) * H],
                                    lhsT=oh[:, k * P:(k + 1) * P],
                                    rhs=ga[:, k * P:(k + 1) * P],
                                    start=not started[bi],
                                    stop=(j == nt_b - 1))
                                started[bi] = True
                                k += 1
                            if not started[bi]:
                                nc.tensor.matmul(
                                    out=agg_ps[:, bi * H:(bi + 1) * H],
                                    lhsT=zeros_t[:, :P], rhs=bc_t[l][:],
                                    start=True, stop=True)
                                started[bi] = True
                        part = partpool.tile([P, SB * H], BF16, tag="pt",
                                             name=f"part{l}_{s}")
                        nc.scalar.activation(out=part[:], in_=agg_ps[:],
                                             func=AF.Copy)
                        partials[s] = part

                if debug and l == 0:
                    gf = bigp.tile([P, 4 * P], F32, tag="gf")
                    nc.vector.tensor_copy(gf[:], ga_tiles[0][:, :4 * P])
                    nc.sync.dma_start(out=d_dbg_g0[:, :], in_=gf[:])
                    nc.sync.dma_start(out=d_dbg_part[:, :], in_=partials[0][:])

                # b-phase: chunk-1 messages + self-loop, combine, relu
                tc.tile_set_cur_wait(3 * l + 2, enable=PHASING)
                for s in range(NSB):
                    blocks = sb_blocks[s]
                    t0 = int(sb_tile0[s])
                    ta = int(sb_ta[s])
                    tt = int(sb_tt[s])
                    tb = tt - ta
                    gb = gb_tiles[s]
                    agg_ps = psA.tile([P, SB * H], F32, tag="A")
                    started = [False] * len(blocks)
                    if tb > 0:
                        eb_e = ebpool.tile([P, maxTb * H], BF16, tag="e")
                        nc.sync.dma_start(
                            out=eb_e[:, :tb * H],
                            in_=e_reg[s][1][:, :tb * H])
                        oh = ohbpool.tile([P, maxTb * P], BF16, tag="oh")
                        nc.vector.tensor_tensor(
                            out=oh[:, :tb * P]
                                .rearrange("p (k c) -> p k c", c=P),
                            in0=iota_t[:].unsqueeze(1).broadcast_to([P, tb, P]),
                            in1=dst_t[:, t0 + ta:t0 + tt].unsqueeze(2)
                                .broadcast_to([P, tb, P]),
                            op=mybir.AluOpType.is_equal)
                        nc.vector.tensor_mul(
                            out=gb[:, :tb * P], in0=gb[:, :tb * P],
                            in1=eb_e[:, :tb * H])
                        k = 0
                        for bi, b in enumerate(blocks):
                            bc0 = b * P
                            for _ in range(int(T[b, 1])):
                                nc.tensor.matmul(
                                    out=agg_ps[:, bi * H:(bi + 1) * H],
                                    lhsT=oh[:, k * P:(k + 1) * P],
                                    rhs=gb[:, k * P:(k + 1) * P],
                                    start=not started[bi], stop=False)
                                started[bi] = True
                                k += 1
                            # self-loop: + t' (scaled to dis^2 t after x dis)
                            nc.tensor.matmul(
                                out=agg_ps[:, bi * H:(bi + 1) * H],
                                lhsT=ident_t[:], rhs=t_t[:, bc0:bc0 + P],
                                start=not started[bi], stop=True)
                            started[bi] = True
                    else:
                        for bi, b in enumerate(blocks):
                            bc0 = b * P
                            nc.tensor.matmul(
                                out=agg_ps[:, bi * H:(bi + 1) * H],
                                lhsT=ident_t[:], rhs=t_t[:, bc0:bc0 + P],
                                start=True, stop=True)
                            started[bi] = True
                    nb_sb = len(blocks)
                    b0 = blocks[0]
                    hsum = small.tile([P, SB * H], BF16, tag="hs")
                    if s in partials:
                        nc.vector.tensor_add(out=hsum[:, :nb_sb * H],
                                             in0=agg_ps[:, :nb_sb * H],
                                             in1=partials[s][:, :nb_sb * H])
                    else:
                        nc.vector.tensor_copy(hsum[:, :nb_sb * H],
                                              agg_ps[:, :nb_sb * H])
                    # x dis[dst-node], + bias, relu
                    nc.vector.tensor_tensor(
                        out=hsum[:, :nb_sb * H]
                            .rearrange("p (b c) -> p b c", c=H),
                        in0=hsum[:, :nb_sb * H]
                            .rearrange("p (b c) -> p b c", c=H),
                        in1=dis_t[:, b0:b0 + nb_sb].unsqueeze(2)
                            .broadcast_to([P, nb_sb, H]),
                        op=mybir.AluOpType.mult)
                    nc.vector.tensor_tensor(
                        out=hsum[:, :nb_sb * H]
                            .rearrange("p (b c) -> p b c", c=H),
                        in0=hsum[:, :nb_sb * H]
                            .rearrange("p (b c) -> p b c", c=H),
                        in1=bcb_t[:, l * H:(l + 1) * H].unsqueeze(1)
                            .broadcast_to([P, nb_sb, H]),
                        op=mybir.AluOpType.add)
                    nc.scalar.activation(
                        out=h_t[:, b0 * P:(b0 + nb_sb) * P],
                        in_=hsum[:, :nb_sb * H], func=AF.Relu)

            if debug:
                nc.sync.dma_start(out=d_dbg_h1[:, :], in_=h_t[:])
                for s in range(NSB):
                    ta = int(sb_ta[s])
                    tb = int(sb_tt[s]) - ta
                    t0 = int(sb_tile0[s])
                    for (rh, rn) in ((0, ta), (1, tb)):
                        if rn == 0:
                            continue
                        roff = t0 if rh == 0 else t0 + ta
                        nc.sync.dma_start(
                            out=d_dbg_e[:, roff * H:(roff + rn) * H],
                            in_=e_reg[s][rh][:, :rn * H])

            # ---- global mean pool ----
            tc.tile_set_cur_wait(3 * L, enable=PHASING)
            ohp_t = ohapool.tile([P, NB * gpc], BF16, tag="oh", name="ohp_all")
            nc.vector.tensor_tensor(
                out=ohp_t[:].rearrange("p (b c) -> p b c", c=gpc),
                in0=iota32_t[:].unsqueeze(1).broadcast_to([P, NB, gpc]),
                in1=batch_t[:].unsqueeze(2).broadcast_to([P, NB, gpc]),
                op=mybir.AluOpType.is_equal)
            g_ps = psT.tile([gpc, H], F32, tag="T")
            for b in range(NB):
                nc.tensor.matmul(out=g_ps[:],
                                 lhsT=ohp_t[:, b * gpc:(b + 1) * gpc],
                                 rhs=h_t[:, b * P:(b + 1) * P],
                                 start=(b == 0), stop=(b == NB - 1))
            g_sb = small.tile([gpc, H], F32, tag="gsb")
            nc.vector.tensor_scalar(out=g_sb[:], in0=g_ps[:],
                                    scalar1=invc_t[:gpc, 0:1], scalar2=None,
                                    op0=mybir.AluOpType.mult)

            # ---- head ----
            gT_ps = psT.tile([P, gpc], F32, tag="T")
            nc.tensor.transpose(out=gT_ps[:], in_=g_sb[:],
                                identity=identf_t[:gpc, :gpc])
            gT_sb = small.tile([P, gpc], F32, tag="gT")
            nc.vector.tensor_copy(gT_sb[:], gT_ps[:])
            z1_ps = psT.tile([gpc, H], F32, tag="T")
            nc.tensor.matmul(out=z1_ps[:], lhsT=gT_sb[:], rhs=Wl1_t[:],
                             start=True, stop=False)
            nc.tensor.matmul(out=z1_ps[:], lhsT=ones_t[:, :gpc], rhs=bl1_t[:],
                             start=False, stop=True)
            z1_sb = small.tile([gpc, H], F32, tag="z1")
            nc.scalar.activation(out=z1_sb[:], in_=z1_ps[:], func=AF.Relu)
            z1T_ps = psT.tile([P, gpc], F32, tag="T")
            nc.tensor.transpose(out=z1T_ps[:], in_=z1_sb[:],
                                identity=identf_t[:gpc, :gpc])
            z1T_sb = small.tile([P, gpc], F32, tag="z1T")
            nc.vector.tensor_copy(z1T_sb[:], z1T_ps[:])
            o2_ps = psT.tile([gpc, 1], F32, tag="T")
            nc.tensor.matmul(out=o2_ps[:], lhsT=z1T_sb[:], rhs=Wl2_t[:],
                             start=True, stop=True)
            out_sb = small.tile([gpc, 1], F32, tag="osb")
            nc.vector.tensor_scalar(out=out_sb[:], in0=o2_ps[:],
                                    scalar1=b_l2_val, scalar2=None,
                                    op0=mybir.AluOpType.add)
            nc.sync.dma_start(out=d_out[:, :], in_=out_sb[:])

    nc.compile()
    return nc


def make_in_maps(plan, weights, n_layers):
    L = n_layers
    iota128 = np.tile(np.arange(P, dtype=np.float32), (P, 1)).astype(NPBF16)
    iota32 = np.tile(np.arange(plan["gpc"], dtype=np.float32),
                     (P, 1)).astype(NPBF16)
    ident = np.eye(P, dtype=np.float32)
    w = {k: np.asarray(v, np.float32) for k, v in weights.items()}
    shared = dict(
        iota128=iota128, iota32=np.ascontiguousarray(iota32),
        identity=ident.astype(NPBF16), identityf=ident,
        W_node=w["W_node"].astype(NPBF16), b_node=w["b_node"].reshape(1, H),
        W_e1=w["W_e1"].astype(NPBF16), b_e1=w["b_e1"].reshape(H, 1),
        W_e2=w["W_e2"].astype(NPBF16), b_e2=w["b_e2"].reshape(1, H),
        W_convs=w["W_convs"].reshape(L, H, H).astype(NPBF16),
        b_convs=w["b_convs"].reshape(L, H),
        b_convs_bcast=np.tile(w["b_convs"].reshape(1, L * H), (P, 1)),
        W_l1=w["W_l1"], b_l1=w["b_l1"].reshape(1, H),
        W_l2=w["W_l2"].reshape(H, 1),
    )
    in_maps = []
    for c in range(NC):
        m = dict(shared)
        cc = plan["cores"][c]
        m.update(eaT=cc["eaT"], dst_col=cc["dst_col"],
                 idx16=cc["idx16"], xT=cc["xT"], dis_col=cc["dis_col"],
                 batch_col=cc["batch_col"], invc=cc["invc"])
        in_maps.append(m)
    return in_maps


# ----------------------------------------------------------------------------
# Public entry: kernel(**inputs) -> [256, 1] float32
# ----------------------------------------------------------------------------
N_GRAPHS = 256
N_LAYERS = 3


def _build_for_inputs(inputs):
    plan = preprocess(inputs["x"], inputs["edge_attr"], inputs["edge_index"],
                      inputs["batch"], N_GRAPHS)
    wkeys = ["W_node", "b_node", "W_e1", "b_e1", "W_e2", "b_e2", "W_convs",
             "b_convs", "W_l1", "b_l1", "W_l2", "b_l2"]
    w = {k: np.asarray(inputs[k], np.float32) for k in wkeys}
    nc_ = build_kernel(plan, w, N_LAYERS)
    in_maps = make_in_maps(plan, w, N_LAYERS)
    return nc_, in_maps, plan


def kernel(**inputs):
    from concourse.bass_utils import run_bass_kernel_spmd
    inputs = {k: np.asarray(v) for k, v in inputs.items()}
    nc_, in_maps, plan = _build_for_inputs(inputs)
    res = run_bass_kernel_spmd(nc_, in_maps, core_ids=list(range(NC)))
    out = np.concatenate([res.results[c]["out"] for c in range(NC)], axis=0)
    return out.astype(np.float32)
